# revision 38
# baseline (speedup 1.0000x reference)
"""Chamfer distance kernel for Trainium2, 8 NeuronCores.

Math: dist2[m, n] = |y_m|^2 + |x_n|^2 - 2 y_m.x_n, computed as ONE K=16
matmul per tile using a bf16 hi/lo split of every operand (all 4 cross
terms kept), accumulated in fp32 PSUM -> ~1e-5 relative accuracy.
min(sqrt(d)) == sqrt(min(d)), so all mins run on squared distances and the
sqrt happens on the host over just B*(M+N) values.

Sharding: core c handles batch b = c//2, y-half h = c%2 (2048 of 4096 y
rows), all 4096 x rows.  Single pass over D2 tiles [128 m, 2048 n]:
  ScalarE cast-copies each PSUM tile to SBUF fp16 (values pre-scaled by
  256 so fp16 never goes subnormal); every min then runs on the DVE in
  fp16, which hits the 2x_1P perf mode (2 elements/cycle) that fp32
  tensor ops can't reach:
  rowmin: fold min(ct0, ct1) -> 2048 -> 1024 wide, one [128, 1024] stripe
      per m-block (host finishes the 1024-way min from the DMA'd stripes)
  colmin: running tensor_tensor(min) into an SBUF fp16 accumulator
      [128, 4096]; lane p holds min over m = i*128+p.  DMA'd out; the
      128-lane + core-half reduction happens on host.
fp16 quantization (2^-11 relative, on top of the ~2^-24 matmul path) is
zero-mean across the 32k independent min values and changes the final
mean by <1e-5 relative (verified against the jax reference).
"""

import numpy as np
import ml_dtypes

_B, _N, _M, _D = 4, 4096, 4096, 3
_MHALF = _M // 2
_NCORES = 8
_K = 24                  # 3-way bf16 split of [ones|norm|(-2y_d)] x [norm|ones|x_d]
_SCALE = 16.0            # per side; D2 carries x256 so fp16 mins stay normal
_BIG = 3.0e38

_cache = {}


def _bf16_3split(v):
    """fp32 array -> 3 bf16 parts with v ~= p0 + p1 + p2 (24 mantissa bits)."""
    v = v.astype(np.float32)
    a = v.astype(ml_dtypes.bfloat16)
    r = v - a.astype(np.float32)
    b = r.astype(ml_dtypes.bfloat16)
    c = (r - b.astype(np.float32)).astype(ml_dtypes.bfloat16)
    return [a, b, c]


# product split terms (i, j) with i+j <= 2: error floor ~2^-24 per product
_PAIR_IJ = [(0, 0), (0, 1), (1, 0), (0, 2), (2, 0), (1, 1)]


def _side_matrices(xb, yb):
    """Return (ya [24, M'], xa [24, N]) bf16 for one (batch, y-half).

    sum_k ya[k, m] * xa[k, n] ~= |y_m|^2 + |x_n|^2 - 2 y_m.x_n to ~2^-24,
    using a 3-way bf16 split of every operand:
      k0-2 : ones      <-> xnorm parts      k3-5 : ynorm parts <-> ones
      per d: (-2y_d)_i <-> (x_d)_j for (i, j) in _PAIR_IJ
    """
    n = xb.shape[0]
    m = yb.shape[0]
    xb = np.ascontiguousarray(xb, np.float32)
    yb = np.ascontiguousarray(yb, np.float32)
    xnorm = np.einsum("nd,nd->n", xb, xb, dtype=np.float32, optimize=True)
    ynorm = np.einsum("md,md->m", yb, yb, dtype=np.float32, optimize=True)
    t = (-2.0 * yb).astype(np.float32)
    ones_x = np.ones(n, ml_dtypes.bfloat16)
    ones_y = np.ones(m, ml_dtypes.bfloat16)
    ya_rows, xa_rows = [], []
    for part in _bf16_3split(xnorm):
        ya_rows.append(ones_y)
        xa_rows.append(part)
    for part in _bf16_3split(ynorm):
        ya_rows.append(part)
        xa_rows.append(ones_x)
    for d in range(_D):
        ts = _bf16_3split(t[:, d])
        xs = _bf16_3split(xb[:, d])
        for i, j in _PAIR_IJ:
            ya_rows.append(ts[i])
            xa_rows.append(xs[j])
    ya = np.stack(ya_rows).astype(np.float32) * _SCALE
    xa = np.stack(xa_rows).astype(np.float32) * _SCALE
    ya = np.ascontiguousarray(ya, dtype=ml_dtypes.bfloat16)
    xa = np.ascontiguousarray(xa, dtype=ml_dtypes.bfloat16)
    assert ya.shape[0] == _K
    return ya, xa


def _split_excess_waits(nc, mybir, maxw=1):
    """This walrus build accepts only one sync-wait per instruction; hoist
    extra waits onto wait-only Drain instructions inserted just before the
    over-limit instruction on the same engine.  (A wait-only EventSemaphore
    looks cheaper but wedges the device — empirically it must carry an
    update; Drain is safe.)"""
    n_split = 0
    for f in nc.m.functions:
        for b in f.blocks:
            il = b.instructions
            idx = 0
            while idx < len(il):
                ins = il[idx]
                si = ins.sync_info
                if si is not None and len(si.on_wait) > maxw:
                    waits = list(si.on_wait)
                    keep = waits[-maxw:]
                    extra = waits[:-maxw]
                    ins.sync_info = mybir.SyncInfo(
                        on_wait=keep, on_update=list(si.on_update)
                    )
                    for j in range(0, len(extra), maxw):
                        d = mybir.InstDrain(
                            name=f"{ins.name}-wsplit{j}",
                            engine=ins.engine,
                            ins=[],
                            outs=[],
                            sync_info=mybir.SyncInfo(
                                on_wait=extra[j : j + maxw], on_update=[]
                            ),
                        )
                        il.insert(idx, d)
                        idx += 1
                    n_split += 1
                idx += 1
    return n_split


def build_bass(loop_n=1):
    """Build the single SPMD Bass module (same program on all 8 cores).

    loop_n > 1 wraps the compute body in an on-device For_i that repeats the
    (idempotent) min accumulation — used by test.py to measure the per
    -iteration hardware time without RPC noise."""
    import contextlib
    import concourse.bass as bass
    import concourse.tile as tile
    from concourse import mybir

    MIN = mybir.AluOpType.min
    f32 = mybir.dt.float32
    bf16 = mybir.dt.bfloat16
    fp16 = mybir.dt.float16

    nc = bass.Bass(trn_type="TRN2")
    ya_d = nc.dram_tensor("ya", [_K, _MHALF], bf16, kind="ExternalInput")
    xa_d = nc.dram_tensor("xa", [_K, _N], bf16, kind="ExternalInput")
    n_mblk = _MHALF // 128          # 16
    TW = 2048                       # psum tile free width (4 banks)
    RW = 2048                       # rowmin stripe width per m-block
    rowf_d = nc.dram_tensor("rowf", [128, n_mblk * RW], fp16, kind="ExternalOutput")
    colacc_d = nc.dram_tensor("colacc", [128, _N], fp16, kind="ExternalOutput")

    with tile.TileContext(nc) as tc:
        with (
            tc.tile_pool(name="inputs", bufs=1) as inputs,
            tc.tile_pool(name="outs", bufs=1) as outs,
            tc.tile_pool(name="cts", bufs=4) as cts,
            tc.tile_pool(name="folds", bufs=2) as folds,
            tc.tile_pool(name="psum", bufs=2, space="PSUM") as psum,
        ):
            yr = inputs.tile([128, _MHALF], bf16)
            xr = inputs.tile([128, _N], bf16)
            nc.sync.dma_start(out=yr[:_K, :], in_=ya_d[:, :])
            nc.sync.dma_start(out=xr[:_K, :], in_=xa_d[:, :])

            # ping-pong accumulators: writing min(acc, ct) to the *other*
            # buffer keeps the DVE tensor_tensor out of the in-place
            # read-modify-write path
            colaccs = [
                outs.tile([128, _N], fp16, name=f"colacc{z}", tag=f"colacc{z}")
                for z in range(2)
            ]
            colacc = colaccs[0]
            nc.vector.memset(colaccs[0], 60000.0)

            loop_cm = contextlib.ExitStack()
            if loop_n > 1:
                loop_cm.enter_context(tc.For_i(0, loop_n, 1))

            for i in range(n_mblk):
                acc_src = colaccs[i % 2]
                acc_dst = colaccs[(i + 1) % 2]
                ct_pair = []
                for j in range(2):
                    pt = psum.tile([128, TW], f32)
                    for q in range(4):
                        c0 = j * TW + q * 512
                        nc.tensor.matmul(
                            pt[:, q * 512 : (q + 1) * 512],
                            lhsT=yr[:_K, i * 128 : (i + 1) * 128],
                            rhs=xr[:_K, c0 : c0 + 512],
                            start=True,
                            stop=True,
                        )
                    ct = cts.tile([128, TW], fp16)
                    nc.scalar.copy(out=ct[:, :], in_=pt[:, :])
                    nc.vector.tensor_tensor(
                        out=acc_dst[:, j * TW : (j + 1) * TW],
                        in0=ct[:, :],
                        in1=acc_src[:, j * TW : (j + 1) * TW],
                        op=MIN,
                    )
                    ct_pair.append(ct)
                rf = folds.tile([128, RW], fp16)
                nc.vector.tensor_tensor(
                    out=rf[:, :],
                    in0=ct_pair[0][:, :],
                    in1=ct_pair[1][:, :],
                    op=MIN,
                )
                # stream each block's rowmin stripe out while compute runs
                nc.sync.dma_start(
                    out=rowf_d[:, i * RW : (i + 1) * RW], in_=rf[:, :]
                )

            loop_cm.close()
            nc.sync.dma_start(out=colacc_d[:, :], in_=colacc[:, :])

    _split_excess_waits(nc, mybir)
    return nc


def _get_nc():
    if "nc" not in _cache:
        _cache["nc"] = build_bass()
    return _cache["nc"]


def make_in_maps(x, y):
    """Per-core input dicts: core c -> (batch c//2, y-half c%2)."""
    x = np.asarray(x, dtype=np.float32)
    y = np.asarray(y, dtype=np.float32)
    in_maps = []
    for c in range(_NCORES):
        b, h = divmod(c, 2)
        ya, xa = _side_matrices(x[b], y[b, h * _MHALF : (h + 1) * _MHALF])
        in_maps.append({"ya": ya, "xa": xa})
    return in_maps


def reduce_outputs(results):
    """Host-side gather: per-core mins -> final scalar."""
    inv = 1.0 / (_SCALE * _SCALE)
    d2_m = np.empty((_B, _M), np.float64)
    d2_n = np.full((_B, _N), np.inf, np.float64)
    for c, r in enumerate(results):
        b, h = divmod(c, 2)
        rf = np.asarray(r["rowf"]).astype(np.float64)   # [128, 16*RW]
        rm_blk = rf.reshape(128, 16, -1).min(axis=2)    # [128, 16]; m = i*128+p
        d2_m[b, h * _MHALF : (h + 1) * _MHALF] = rm_blk.T.reshape(-1) * inv
        ca = np.asarray(r["colacc"]).astype(np.float64)  # [128, 4096]
        np.minimum(d2_n[b], ca.min(axis=0) * inv, out=d2_n[b])
    mean_m = np.sqrt(np.maximum(d2_m, 0.0)).mean()
    mean_n = np.sqrt(np.maximum(d2_n, 0.0)).mean()
    return np.float32(mean_m + mean_n)


def kernel(x, y):
    from concourse.bass_utils import run_bass_kernel_spmd

    nc = _get_nc()
    in_maps = make_in_maps(x, y)
    res = run_bass_kernel_spmd(nc, in_maps, core_ids=list(range(_NCORES)))
    return reduce_outputs(res.results)


# revision 40
# speedup vs baseline: 1.1026x; 1.1026x over previous
"""Chamfer distance kernel for Trainium2, 8 NeuronCores.

Math: dist2[m, n] = |y_m|^2 + |x_n|^2 - 2 y_m.x_n, computed as ONE K=16
matmul per tile using a bf16 hi/lo split of every operand (all 4 cross
terms kept), accumulated in fp32 PSUM -> ~1e-5 relative accuracy.
min(sqrt(d)) == sqrt(min(d)), so all mins run on squared distances and the
sqrt happens on the host over just B*(M+N) values.

Sharding: core c handles batch b = c//2, y-half h = c%2 (2048 of 4096 y
rows), all 4096 x rows.  Single pass over D2 tiles [128 m, 2048 n]:
  ScalarE cast-copies each PSUM tile to SBUF fp16 (values pre-scaled by
  256 so fp16 never goes subnormal); every min then runs on the DVE in
  fp16, which hits the 2x_1P perf mode (2 elements/cycle) that fp32
  tensor ops can't reach:
  rowmin: fold min(ct0, ct1) -> 2048 -> 1024 wide, one [128, 1024] stripe
      per m-block (host finishes the 1024-way min from the DMA'd stripes)
  colmin: running tensor_tensor(min) into an SBUF fp16 accumulator
      [128, 4096]; lane p holds min over m = i*128+p.  DMA'd out; the
      128-lane + core-half reduction happens on host.
fp16 quantization (2^-11 relative, on top of the ~2^-24 matmul path) is
zero-mean across the 32k independent min values and changes the final
mean by <1e-5 relative (verified against the jax reference).
"""

import numpy as np
import ml_dtypes

_B, _N, _M, _D = 4, 4096, 4096, 3
_MHALF = _M // 2
_NCORES = 8
_K = 24                  # 3-way bf16 split of [ones|norm|(-2y_d)] x [norm|ones|x_d]
_SCALE = 16.0            # per side; D2 carries x256 so fp16 mins stay normal
_BIG = 3.0e38

_cache = {}


def _bf16_3split(v):
    """fp32 array -> 3 bf16 parts with v ~= p0 + p1 + p2 (24 mantissa bits)."""
    v = v.astype(np.float32)
    a = v.astype(ml_dtypes.bfloat16)
    r = v - a.astype(np.float32)
    b = r.astype(ml_dtypes.bfloat16)
    c = (r - b.astype(np.float32)).astype(ml_dtypes.bfloat16)
    return [a, b, c]


# product split terms (i, j) with i+j <= 2: error floor ~2^-24 per product
_PAIR_IJ = [(0, 0), (0, 1), (1, 0), (0, 2), (2, 0), (1, 1)]


def _side_matrices(xb, yb):
    """Return (ya [24, M'], xa [24, N]) bf16 for one (batch, y-half).

    sum_k ya[k, m] * xa[k, n] ~= |y_m|^2 + |x_n|^2 - 2 y_m.x_n to ~2^-24,
    using a 3-way bf16 split of every operand:
      k0-2 : ones      <-> xnorm parts      k3-5 : ynorm parts <-> ones
      per d: (-2y_d)_i <-> (x_d)_j for (i, j) in _PAIR_IJ
    """
    n = xb.shape[0]
    m = yb.shape[0]
    xb = np.ascontiguousarray(xb, np.float32)
    yb = np.ascontiguousarray(yb, np.float32)
    xnorm = np.einsum("nd,nd->n", xb, xb, dtype=np.float32, optimize=True)
    ynorm = np.einsum("md,md->m", yb, yb, dtype=np.float32, optimize=True)
    t = (-2.0 * yb).astype(np.float32)
    ones_x = np.ones(n, ml_dtypes.bfloat16)
    ones_y = np.ones(m, ml_dtypes.bfloat16)
    ya_rows, xa_rows = [], []
    for part in _bf16_3split(xnorm):
        ya_rows.append(ones_y)
        xa_rows.append(part)
    for part in _bf16_3split(ynorm):
        ya_rows.append(part)
        xa_rows.append(ones_x)
    for d in range(_D):
        ts = _bf16_3split(t[:, d])
        xs = _bf16_3split(xb[:, d])
        for i, j in _PAIR_IJ:
            ya_rows.append(ts[i])
            xa_rows.append(xs[j])
    ya = np.stack(ya_rows).astype(np.float32) * _SCALE
    xa = np.stack(xa_rows).astype(np.float32) * _SCALE
    ya = np.ascontiguousarray(ya, dtype=ml_dtypes.bfloat16)
    xa = np.ascontiguousarray(xa, dtype=ml_dtypes.bfloat16)
    assert ya.shape[0] == _K
    return ya, xa


def _split_excess_waits(nc, mybir, maxw=1):
    """This walrus build accepts only one sync-wait per instruction; hoist
    extra waits onto wait-only Drain instructions inserted just before the
    over-limit instruction on the same engine.  (A wait-only EventSemaphore
    looks cheaper but wedges the device — empirically it must carry an
    update; Drain is safe.)"""
    n_split = 0
    for f in nc.m.functions:
        for b in f.blocks:
            il = b.instructions
            idx = 0
            while idx < len(il):
                ins = il[idx]
                si = ins.sync_info
                if si is not None and len(si.on_wait) > maxw:
                    waits = list(si.on_wait)
                    keep = waits[-maxw:]
                    extra = waits[:-maxw]
                    ins.sync_info = mybir.SyncInfo(
                        on_wait=keep, on_update=list(si.on_update)
                    )
                    for j in range(0, len(extra), maxw):
                        d = mybir.InstDrain(
                            name=f"{ins.name}-wsplit{j}",
                            engine=ins.engine,
                            ins=[],
                            outs=[],
                            sync_info=mybir.SyncInfo(
                                on_wait=extra[j : j + maxw], on_update=[]
                            ),
                        )
                        il.insert(idx, d)
                        idx += 1
                    n_split += 1
                idx += 1
    return n_split


def build_bass(loop_n=1):
    """Build the single SPMD Bass module (same program on all 8 cores).

    loop_n > 1 wraps the compute body in an on-device For_i that repeats the
    (idempotent) min accumulation — used by test.py to measure the per
    -iteration hardware time without RPC noise."""
    import contextlib
    import concourse.bass as bass
    import concourse.tile as tile
    from concourse import mybir

    MIN = mybir.AluOpType.min
    f32 = mybir.dt.float32
    bf16 = mybir.dt.bfloat16
    fp16 = mybir.dt.float16

    nc = bass.Bass(trn_type="TRN2")
    ya_d = nc.dram_tensor("ya", [_K, _MHALF], bf16, kind="ExternalInput")
    xa_d = nc.dram_tensor("xa", [_K, _N], bf16, kind="ExternalInput")
    n_mblk = _MHALF // 128          # 16
    TW = 2048                       # psum tile free width (4 banks)
    RW = 2048                       # rowmin stripe width per m-block
    rowf_d = nc.dram_tensor("rowf", [128, n_mblk * RW], fp16, kind="ExternalOutput")
    colacc_d = nc.dram_tensor("colacc", [128, _N], fp16, kind="ExternalOutput")

    with tile.TileContext(nc) as tc:
        with (
            tc.tile_pool(name="inputs", bufs=1) as inputs,
            tc.tile_pool(name="outs", bufs=1) as outs,
            tc.tile_pool(name="cts", bufs=4) as cts,
            tc.tile_pool(name="folds", bufs=2) as folds,
            tc.tile_pool(name="psum", bufs=2, space="PSUM") as psum,
        ):
            yr = inputs.tile([128, _MHALF], bf16)
            xr = inputs.tile([128, _N], bf16)
            nc.sync.dma_start(out=yr[:_K, :], in_=ya_d[:, :])
            nc.sync.dma_start(out=xr[:_K, :], in_=xa_d[:, :])

            rowf = outs.tile([128, n_mblk * RW], fp16)
            # ping-pong accumulators: writing min(acc, ct) to the *other*
            # buffer keeps the DVE tensor_tensor out of the in-place
            # read-modify-write path
            colaccs = [
                outs.tile([128, _N], fp16, name=f"colacc{z}", tag=f"colacc{z}")
                for z in range(2)
            ]
            colacc = colaccs[0]
            nc.vector.memset(colaccs[0], 60000.0)

            loop_cm = contextlib.ExitStack()
            if loop_n > 1:
                loop_cm.enter_context(tc.For_i(0, loop_n, 1))

            for i in range(n_mblk):
                acc_src = colaccs[i % 2]
                acc_dst = colaccs[(i + 1) % 2]
                ct_pair = []
                for j in range(2):
                    pt = psum.tile([128, TW], f32)
                    for q in range(4):
                        c0 = j * TW + q * 512
                        nc.tensor.matmul(
                            pt[:, q * 512 : (q + 1) * 512],
                            lhsT=yr[:_K, i * 128 : (i + 1) * 128],
                            rhs=xr[:_K, c0 : c0 + 512],
                            start=True,
                            stop=True,
                        )
                    ct = cts.tile([128, TW], fp16)
                    nc.scalar.copy(out=ct[:, :], in_=pt[:, :])
                    nc.vector.tensor_tensor(
                        out=acc_dst[:, j * TW : (j + 1) * TW],
                        in0=ct[:, :],
                        in1=acc_src[:, j * TW : (j + 1) * TW],
                        op=MIN,
                    )
                    ct_pair.append(ct)
                nc.vector.tensor_tensor(
                    out=rowf[:, i * RW : (i + 1) * RW],
                    in0=ct_pair[0][:, :],
                    in1=ct_pair[1][:, :],
                    op=MIN,
                )

            loop_cm.close()
            nc.sync.dma_start(out=rowf_d[:, :], in_=rowf[:, :])
            nc.sync.dma_start(out=colacc_d[:, :], in_=colacc[:, :])

    _split_excess_waits(nc, mybir)
    return nc


def _get_nc():
    if "nc" not in _cache:
        _cache["nc"] = build_bass()
    return _cache["nc"]


def make_in_maps(x, y):
    """Per-core input dicts: core c -> (batch c//2, y-half c%2)."""
    x = np.asarray(x, dtype=np.float32)
    y = np.asarray(y, dtype=np.float32)
    in_maps = []
    for c in range(_NCORES):
        b, h = divmod(c, 2)
        ya, xa = _side_matrices(x[b], y[b, h * _MHALF : (h + 1) * _MHALF])
        in_maps.append({"ya": ya, "xa": xa})
    return in_maps


def reduce_outputs(results):
    """Host-side gather: per-core mins -> final scalar."""
    inv = 1.0 / (_SCALE * _SCALE)
    d2_m = np.empty((_B, _M), np.float64)
    d2_n = np.full((_B, _N), np.inf, np.float64)
    for c, r in enumerate(results):
        b, h = divmod(c, 2)
        rf = np.asarray(r["rowf"]).astype(np.float64)   # [128, 16*RW]
        rm_blk = rf.reshape(128, 16, -1).min(axis=2)    # [128, 16]; m = i*128+p
        d2_m[b, h * _MHALF : (h + 1) * _MHALF] = rm_blk.T.reshape(-1) * inv
        ca = np.asarray(r["colacc"]).astype(np.float64)  # [128, 4096]
        np.minimum(d2_n[b], ca.min(axis=0) * inv, out=d2_n[b])
    mean_m = np.sqrt(np.maximum(d2_m, 0.0)).mean()
    mean_n = np.sqrt(np.maximum(d2_n, 0.0)).mean()
    return np.float32(mean_m + mean_n)


def kernel(x, y):
    from concourse.bass_utils import run_bass_kernel_spmd

    nc = _get_nc()
    in_maps = make_in_maps(x, y)
    res = run_bass_kernel_spmd(nc, in_maps, core_ids=list(range(_NCORES)))
    return reduce_outputs(res.results)
